# revision 6
# baseline (speedup 1.0000x reference)
"""Trainium2 Bass kernel for a GRU encoder-decoder (KLCPD generator).

Model (see reference):
  past_emb = relu(past @ W_emb + b_emb)            [T,B,E]
  fut_emb  = relu(future @ W_emb + b_emb)          [T,B,E]
  _, h_T   = GRU_enc(past_emb, h0=0)
  hidden   = h_T + noise
  ys, _    = GRU_dec(shift(fut_emb), h0=hidden)
  out      = ys @ W_out + b_out                    [T,B,D]

Sharding: data-parallel over batch B=1024 across 8 NeuronCores
(B_local=128); all weights replicated; no collectives.

Per-core kernel layout (batch-major / moving-weights scheme):
  * The previous scheme kept the state transposed and used the GRU
    weights as the PE's stationary operand: 72 matmuls per step with
    only N=128 moving columns (the per-core batch).  Measured HW
    profile: 9.6k matmuls at ~79ns each (LDWEIGHTS-bound), PE busy
    82%, 762us total.
  * This scheme flips the operands: the STATE is the stationary
    operand (hT chunks, 4 LDWs/step) and the weights are the MOVING
    operand (W_hh[k] rows streamed as N=512 slices).  12 recurrent +
    6 input-projection matmuls per step, all N=512, running at the
    pure streaming rate (~216ns each) instead of the LDW-bound rate.
  * Gate pre-activations land batch-major ([batch, 3H]) in PSUM:
    banks r,z double-buffered (4), xn single (1), hn single (1),
    h-transpose (1), emb/output-projection shared slot (1) = 8 banks.
  * The tail (sigmoid/tanh/elementwise) is unchanged except that it
    runs batch-major and each 128-col quarter of h_new is immediately
    PE-transposed (bf16 identity) and copied back to SBUF to form the
    next step's stationary hT chunks.
  * Embeddings stream through the loops as before (slot-batched input
    DMAs, PE transposes + matmuls through the shared PSUM bank); embT
    chunks now serve as the stationary operand of the input
    projections with W_ih moving.
  * The decoder output projection reuses the hT chunks as stationary
    (po[b,d] = h @ W_out directly); 4 steps batch into one output DMA.
  * noise no longer needs a transpose (hidden = h_T + noise is
    batch-major).
"""

import os
from contextlib import ExitStack

import numpy as np

import concourse.bass as bass
import concourse.tile as tile
from concourse import bacc, bass_utils, masks, mybir
from concourse.tile_rust import add_dep_helper

T, B, D, E, H = 64, 1024, 128, 256, 512
NCORES = 8
BL = B // NCORES  # 128
H3 = 3 * H
P = 128

f32 = mybir.dt.float32
bf16 = mybir.dt.bfloat16
AF = mybir.ActivationFunctionType
OP = mybir.AluOpType


def _mm(nc, out, lhsT, rhs, start, stop):
    nc.tensor.matmul(out, lhsT, rhs, start=start, stop=stop, skip_group_check=True)


# Tunables.
CFG = {
    "tail_halves": 2,     # 1 = full-width gate ops, 2 = H-halved
    "w_on_gpsimd": False,  # offload w = z*h to the GpSimd engine
    "emb_all_pre": False,  # emit all embedding groups before the enc loop
    "emb_lead": 1,         # steps between emb stage A and stage B
    "emb_e1_gpsimd": False,  # e1 relu on gpsimd instead of DVE
    "emb_cadence": 2,        # steps between embedding-group stage As
    "tail_first_cols": 256,  # width of the first (chain-critical) tail slice
    "po_direct_dma": False,  # accumulate 4 steps in one PSUM bank, DMA direct
    "copy_dve": False,       # hT quarter copies on DVE instead of ACT
}
for _k in list(CFG):
    _v = os.environ.get(f"KCFG_{_k.upper()}")
    if _v is not None:
        CFG[_k] = type(CFG[_k])(int(_v))


def build_module(zero_bias: bool, t_steps: int = T, dump_h: bool = False):
    """Builds the per-core Bass module. Returns the compiled nc."""
    nc = bacc.Bacc("TRN2", target_bir_lowering=False, debug=False)
    dbg_h = None
    if dump_h:
        dbg_h = nc.dram_tensor("dbg_h", [2, t_steps, P, H], bf16, kind="ExternalOutput").ap()

    past = nc.dram_tensor("past", [t_steps, BL, D], f32, kind="ExternalInput").ap()
    fut = nc.dram_tensor("fut", [t_steps, BL, D], f32, kind="ExternalInput").ap()
    noise = nc.dram_tensor("noise", [BL, H], f32, kind="ExternalInput").ap()
    w_emb = nc.dram_tensor("w_emb", [D, E], f32, kind="ExternalInput").ap()
    b_emb = nc.dram_tensor("b_emb", [1, E], f32, kind="ExternalInput").ap()
    wd = {}
    for g in ("enc", "dec"):
        wd[g, "ih"] = nc.dram_tensor(f"w_ih_{g}", [E, H3], f32, kind="ExternalInput").ap()
        wd[g, "hh"] = nc.dram_tensor(f"w_hh_{g}", [H, H3], f32, kind="ExternalInput").ap()
        wd[g, "bih"] = nc.dram_tensor(f"b_ih_{g}", [1, H3], f32, kind="ExternalInput").ap()
        wd[g, "bhh"] = nc.dram_tensor(f"b_hh_{g}", [1, H3], f32, kind="ExternalInput").ap()
    w_out = nc.dram_tensor("w_out", [H, D], f32, kind="ExternalInput").ap()
    b_out = nc.dram_tensor("b_out", [1, D], f32, kind="ExternalInput").ap()
    out = nc.dram_tensor("out", [t_steps, BL, D], f32, kind="ExternalOutput").ap()

    with tile.TileContext(nc, pool_alloc_mode="queue") as tc, ExitStack() as octx:
        wpool = octx.enter_context(tc.tile_pool(name="weights", bufs=1))

        # ---- constants -------------------------------------------------
        ident = wpool.tile([P, P], f32)
        masks.make_identity(nc, ident[:])
        ident_bf = wpool.tile([P, P], bf16)
        masks.make_identity(nc, ident_bf[:])
        ones_row = wpool.tile([1, 512], bf16)
        nc.gpsimd.memset(ones_row[:], 1.0)

        # ---- embedding precompute (streamed) ----------------------------
        # embT[g][e][p, t*BL + b] = relu(x[t] @ W_emb + b_emb)[b, e*128+p]
        embT = {g: [wpool.tile([P, t_steps * BL], bf16, name=f"embT_{g}_{e}", tag=f"embT_{g}_{e}")
                    for e in range(2)]
                for g in ("enc", "dec")}
        n_grp = t_steps // 4
        ep = octx.enter_context(tc.tile_pool(
            name="estage", bufs=int(os.environ.get("KCFG_EP_BUFS", 3 if zero_bias else 1))))
        pgo = octx.enter_context(tc.tile_pool(name="psum_out", bufs=1, space="PSUM"))

        # Slot-batched input loads: ONE DMA covers 4 embedding groups (16
        # timesteps), amortizing the ~625ns per-DMA engine-queue cost.
        GRP_PER_SLOT = int(os.environ.get("KCFG_GRP_PER_SLOT", 4))
        xs_slots = {}

        def emit_xs_slot(g, x_ap, si, split=False):
            lo = si * GRP_PER_SLOT * 4
            hi = min(lo + GRP_PER_SLOT * 4, t_steps)
            xs = ep.tile([P, (hi - lo) * P], f32, tag="xs")
            halves = ((lo, (lo + hi) // 2), ((lo + hi) // 2, hi)) if split else ((lo, hi),)
            for (a, b) in halves:
                nc.sync.dma_start(
                    xs[:, (a - lo) * P:(b - lo) * P].rearrange("p (i d) -> p i d", i=b - a),
                    x_ap[a:b].transpose([1, 0, 2]),
                )
            xs_slots[g, si] = xs

        def emit_emb_stage_a(g, x_ap, gi):
            """Transpose one 4-step group (PE transposes via the shared PSUM
            bank). Returns a closure emitting stage B (matmul + relu)."""
            si, sub = divmod(gi, GRP_PER_SLOT)
            if (g, si) not in xs_slots:
                emit_xs_slot(g, x_ap, si,
                             split=os.environ.get("KCFG_SPLIT_ALL") == "1")
            xs = xs_slots[g, si]
            ptr = pgo.tile([P, 4 * P], f32, tag="po")
            for i in range(4):
                nc.tensor.transpose(ptr[:, i * P:(i + 1) * P],
                                    xs[:, (sub * 4 + i) * P:(sub * 4 + i + 1) * P],
                                    ident[:])
            xT = ep.tile([P, 4 * P], bf16, tag="xT")
            nc.scalar.copy(xT[:], ptr[:])

            def stage_b():
                for e in range(2):
                    pe_ = pgo.tile([P, 4 * P], f32, tag="po")
                    _mm(nc, pe_[:], wemb_bf[:, e * P:(e + 1) * P], xT[:],
                        start=True, stop=zero_bias)
                    if not zero_bias:
                        _mm(nc, pe_[:], bemb_bf[0:1, e * P:(e + 1) * P], ones_row[0:1, :],
                            start=False, stop=True)
                    dst = embT[g][e][:, gi * 4 * P:(gi + 1) * 4 * P]
                    if e == 0:
                        nc.scalar.activation(dst, pe_[:], AF.Relu)
                    elif CFG["emb_e1_gpsimd"]:
                        nc.gpsimd.tensor_scalar_max(dst, pe_[:], 0.0)
                    else:
                        nc.vector.tensor_scalar_max(dst, pe_[:], 0.0)

            return stage_b

        # Highest priority on the sync DMA ring: the first past-input slot.
        emit_xs_slot("enc", past, 0, split=True)

        # ---- weight preload + cast to bf16 -----------------------------
        whh = {}   # whh[g][k]: [128, H3]  (rows k*128..) -- MOVING operand
        wih = {}   # wih[g][e]: [128, H3]  (rows e*128..) -- MOVING operand
        biasx = {}  # [1, H3]  (b_ih + b_hh on r,z cols; b_ih on n cols)
        biashn = {}  # [1, 512] (b_hh n-part)
        stage_ctx = tc.tile_pool(name="wstage", bufs=2)
        stage = octx.enter_context(stage_ctx)

        wemb_bf = wpool.tile([P, E], bf16)
        st = stage.tile([P, E], f32, tag="s_emb")
        nc.sync.dma_start(st[:], w_emb[:, :])
        nc.vector.tensor_copy(wemb_bf[:], st[:])

        bemb_bf = None
        if not zero_bias:
            st = stage.tile([1, E], f32, tag="s_bemb")
            nc.sync.dma_start(st[:], b_emb[:, :])
            bemb_bf = wpool.tile([1, E], bf16)
            nc.vector.tensor_copy(bemb_bf[:], st[:])

        def load_gru_weights(g, cast_engine=None):
            # Encoder weights ride the scalar HWDGE queue so they do not
            # queue behind the input slots on the sync ring (and vice versa).
            dma_eng = nc.scalar if cast_engine is None else nc.sync
            wih[g] = []
            for e in range(2):
                t_ = wpool.tile([P, H3], bf16, tag=f"wih_{g}_{e}")
                st = stage.tile([P, H3], f32, tag="s_ih")
                dma_eng.dma_start(st[:], wd[g, "ih"][e * P:(e + 1) * P, :])
                if cast_engine is not None:
                    cast_engine.tensor_copy(t_[:], st[:])
                elif e % 2 == 0:
                    nc.vector.tensor_copy(t_[:], st[:])
                else:
                    nc.scalar.copy(t_[:], st[:])
                wih[g].append(t_)
            whh[g] = []
            for k in range(4):
                t_ = wpool.tile([P, H3], bf16, tag=f"whh_{g}_{k}")
                st = stage.tile([P, H3], f32, tag="s_hh")
                dma_eng.dma_start(st[:], wd[g, "hh"][k * P:(k + 1) * P, :])
                if cast_engine is None:
                    if k % 2 == 0:
                        nc.vector.tensor_copy(t_[:], st[:])
                    else:
                        nc.scalar.copy(t_[:], st[:])
                else:
                    cast_engine.tensor_copy(t_[:], st[:])
                whh[g].append(t_)
            if not zero_bias:
                sih = stage.tile([1, H3], f32, tag="s_bih")
                shh = stage.tile([1, H3], f32, tag="s_bhh")
                nc.sync.dma_start(sih[:], wd[g, "bih"][:, :])
                nc.sync.dma_start(shh[:], wd[g, "bhh"][:, :])
                bx = wpool.tile([1, H3], bf16, tag=f"biasx_{g}")
                nc.vector.tensor_add(bx[:, 0:2 * H], sih[:, 0:2 * H], shh[:, 0:2 * H])
                nc.vector.tensor_copy(bx[:, 2 * H:H3], sih[:, 2 * H:H3])
                bh = wpool.tile([1, H], bf16, tag=f"biashn_{g}")
                nc.vector.tensor_copy(bh[:], shh[:, 2 * H:H3])
                biasx[g] = bx
                biashn[g] = bh

        load_gru_weights("enc")

        # noise stays batch-major (hidden = h_T + noise needs no transpose)
        noise_sb = wpool.tile([P, H], bf16)
        st = stage.tile([P, H], f32, tag="s_noise")
        nc.sync.dma_start(st[:], noise[:, :])
        nc.vector.tensor_copy(noise_sb[:], st[:])

        wout_bf = wpool.tile([P, 4 * P], bf16)  # col block k = W_out rows k
        st = stage.tile([P, 4 * P], f32, tag="s_out")
        for k in range(4):
            nc.sync.dma_start(st[:, k * P:(k + 1) * P], w_out[k * P:(k + 1) * P, :])
        nc.vector.tensor_copy(wout_bf[:], st[:])
        if not zero_bias:
            bout_bf = wpool.tile([1, D], bf16)
            st = stage.tile([1, D], f32, tag="s_bout")
            nc.sync.dma_start(st[:], b_out[:, :])
            nc.vector.tensor_copy(bout_bf[:], st[:])

        for _si in range(1, (n_grp + GRP_PER_SLOT - 1) // GRP_PER_SLOT):
            emit_xs_slot("enc", past, _si,
                         split=os.environ.get("KCFG_SPLIT_ALL") == "1")

        N_PRE = n_grp if CFG["emb_all_pre"] else int(os.environ.get("KCFG_N_PRE", 3))
        for gi in range(N_PRE):
            emit_emb_stage_a("enc", past, gi)()
        if CFG["emb_all_pre"]:
            for gi in range(n_grp):
                emit_emb_stage_a("dec", fut, gi)()

        emb_jobs = [("enc", past, gi) for gi in range(N_PRE, n_grp)]
        if not CFG["emb_all_pre"]:
            emb_jobs += [("dec", fut, gi) for gi in range(n_grp)]
        emb_jobs.reverse()  # consume via pop()
        emb_pending = []   # list of (due_step, stage_b)
        emb_clock = [0, -10]  # [global step counter, last stage-A step]

        def emb_filler(t):
            tc_ = emb_clock[0]
            emb_clock[0] += 1
            while emb_pending and emb_pending[0][0] <= tc_:
                emb_pending.pop(0)[1]()
            if (not emb_pending and emb_jobs
                    and tc_ - emb_clock[1] >= CFG["emb_cadence"]):
                emb_clock[1] = tc_
                emb_pending.append((tc_ + CFG["emb_lead"], emit_emb_stage_a(*emb_jobs.pop())))

        # Decoder weights: casts on the otherwise-idle GpSimd engine.
        load_gru_weights("dec", cast_engine=nc.gpsimd)

        # ---- GRU loops --------------------------------------------------
        def gru_loop(g, is_dec, hT0, h0_sb, sb, pg, pgx, pgn, pgt, pgo, extra=None):
            """Runs t_steps of GRU g in batch-major layout.
            hT0 = initial transposed state chunks (SBUF [P, H] bf16,
            hT[p, k*P+b] = h[b, k*P+p]) or None; h0_sb = same state
            batch-major or None. Returns the final batch-major state."""
            hT_prev = hT0
            h_prev = h0_sb

            def emit_xw(t):
                """Allocate step t's r/z/xn PSUM banks and emit the input
                projections (stationary embT chunk, moving W_ih rows)."""
                have_x = (not is_dec) or t > 0
                have_h = t > 0 or hT0 is not None
                have_xn = have_x or not zero_bias
                pr = pg.tile([P, H], f32, name="pr", tag="pr")
                pz = pg.tile([P, H], f32, name="pz", tag="pz")
                pxn = pgx.tile([P, H], f32, name="pxn", tag="pxn") if have_xn else None

                nbias = 0 if zero_bias else 1
                nxw = (2 if have_x else 0) + nbias
                nhw = 4 if have_h else 0
                totals = {id(pr): nxw + nhw, id(pz): nxw + nhw}
                if pxn is not None:
                    totals[id(pxn)] = nxw
                emitted = {k: 0 for k in totals}

                def emit(bank, lhsT, rhs):
                    emitted[id(bank)] += 1
                    _mm(nc, bank[:], lhsT, rhs,
                        start=emitted[id(bank)] == 1,
                        stop=emitted[id(bank)] == totals[id(bank)])

                tcol = (t - 1) if is_dec else t
                if have_x:
                    for e in range(2):
                        lx = embT[g][e][:, tcol * BL:(tcol + 1) * BL]
                        for bank, lo in ((pr, 0), (pz, H), (pxn, 2 * H)):
                            if bank is None:
                                continue
                            emit(bank, lx, wih[g][e][:, lo:lo + H])
                if not zero_bias:
                    for bank, lo in ((pr, 0), (pz, H), (pxn, 2 * H)):
                        if bank is None:
                            continue
                        emit(bank, ones_row[0:1, 0:P], biasx[g][0:1, lo:lo + H])
                return pr, pz, pxn, totals, emitted

            po_buf = [None]

            def emit_po(t, hT):
                """Output projection of decoder step t: out[t] = h @ W_out
                (+ b_out), with hT chunks stationary and W_out rows moving;
                lands batch-major [b, d]. 4 steps per output DMA."""
                sub = t % 4
                if CFG["po_direct_dma"]:
                    if sub == 0:
                        po_buf[0] = pgo.tile([P, 512], f32, name="po", tag="po")
                    po = po_buf[0]
                    sl = po[:, sub * P:(sub + 1) * P]
                    if not zero_bias:
                        _mm(nc, sl, ones_row[0:1, 0:P], bout_bf[0:1, :],
                            start=sub == 0, stop=False)
                    for k in range(4):
                        _mm(nc, sl, hT[:, k * P:(k + 1) * P], wout_bf[:, k * P:(k + 1) * P],
                            start=zero_bias and sub == 0 and k == 0, stop=k == 3)
                    if sub == 3:
                        outf = sb.tile([P, 4 * P], f32, name="outf", tag="outf")
                        nc.scalar.copy(outf[:], po[:])
                        nc.sync.dma_start(
                            out[t - 3:t + 1].transpose([1, 0, 2]),
                            outf[:].rearrange("p (i d) -> p i d", i=4),
                        )
                    return
                po = pgo.tile([P, 512], f32, name="po", tag="po")
                sl = po[:, 0:P]
                if not zero_bias:
                    _mm(nc, sl, ones_row[0:1, 0:P], bout_bf[0:1, :],
                        start=True, stop=False)
                for k in range(4):
                    _mm(nc, sl, hT[:, k * P:(k + 1) * P], wout_bf[:, k * P:(k + 1) * P],
                        start=zero_bias and k == 0, stop=k == 3)
                if po_buf[0] is None:
                    po_buf[0] = sb.tile([P, 4 * P], f32, name="outf", tag="outf")
                nc.scalar.copy(po_buf[0][:, sub * P:(sub + 1) * P], sl)
                if sub == 3:
                    nc.sync.dma_start(
                        out[t - 3:t + 1].transpose([1, 0, 2]),
                        po_buf[0][:].rearrange("p (i d) -> p i d", i=4),
                    )
                    po_buf[0] = None

            prev_h_out = None
            state = emit_xw(0)
            for t in range(t_steps):
                have_x = (not is_dec) or t > 0
                have_h = hT_prev is not None
                have_xn = have_x or not zero_bias
                pr, pz, pxn, totals, emitted = state

                def emit(bank, lhsT, rhs):
                    emitted[id(bank)] += 1
                    _mm(nc, bank[:], lhsT, rhs,
                        start=emitted[id(bank)] == 1,
                        stop=emitted[id(bank)] == totals[id(bank)])

                # -- recurrent matmuls: stationary hT(t-1) chunk k, moving
                # W_hh rows k. Bank-major (r, hn, z) so the tail's
                # chain-critical sigmoid(r) and r*hn unblock earliest. --
                phn = None
                if have_h:
                    phn = pgn.tile([P, H], f32, name="phn", tag="pn")
                    totals[id(phn)] = 4 + (0 if zero_bias else 1)
                    emitted[id(phn)] = 0
                    for k in range(4):
                        emit(pr, hT_prev[:, k * P:(k + 1) * P],
                             whh[g][k][:, 0:H])
                    if not zero_bias:
                        emit(phn, ones_row[0:1, 0:P], biashn[g][0:1, :])
                    for k in range(4):
                        emit(phn, hT_prev[:, k * P:(k + 1) * P],
                             whh[g][k][:, 2 * H:H3])
                if prev_h_out is not None:
                    # Previous step's output projection: fills the PE while
                    # this step's tail runs.
                    emit_po(t - 1, prev_h_out)
                    prev_h_out = None
                if t + 1 < t_steps:
                    state = emit_xw(t + 1)
                if have_h:
                    for k in range(4):
                        emit(pz, hT_prev[:, k * P:(k + 1) * P],
                             whh[g][k][:, H:2 * H])

                # -- gate math (batch-major), chunked in halves ------------
                r_t = sb.tile([P, H], bf16, name="r_t", tag="r")
                z_t = sb.tile([P, H], bf16, name="z_t", tag="z")
                n_t = sb.tile([P, H], bf16, tag="n")
                p_t = sb.tile([P, H], bf16, tag="p")
                h_new = sb.tile([P, H], bf16, tag="h")
                # Last encoder step: hT unused (hidden-add is batch-major).
                skip_T = (not is_dec) and t == t_steps - 1
                hT_new = None
                ptr_t = None
                if not skip_T:
                    hT_new = sb.tile([P, H], bf16, name="hT_new", tag="hT")
                    ptr_t = pgt.tile([P, H], bf16, name="ptr_t", tag="pt")
                if have_h:
                    t1 = sb.tile([P, H], bf16, tag="t1")
                    t2 = sb.tile([P, H], bf16, name="t2", tag="t2") if have_xn else t1
                    w_t = sb.tile([P, H], bf16, tag="w")

                def emit_transp(qs):
                    if hT_new is None:
                        return
                    nc.tensor.transpose(ptr_t[:, qs], h_new[:, qs], ident_bf[:])
                    if CFG["copy_dve"]:
                        nc.vector.tensor_copy(hT_new[:, qs], ptr_t[:, qs])
                    else:
                        nc.scalar.copy(hT_new[:, qs], ptr_t[:, qs])

                fc = CFG["tail_first_cols"]
                tail_slices = ([slice(0, fc), slice(fc, H)] if CFG["tail_halves"] == 2
                               else [slice(0, H)])
                for hs in tail_slices:
                    nc.scalar.activation(r_t[:, hs], pr[:, hs], AF.Sigmoid)
                    nc.scalar.activation(z_t[:, hs], pz[:, hs], AF.Sigmoid)
                    if have_h:
                        nc.vector.tensor_mul(t1[:, hs], r_t[:, hs], phn[:, hs])
                        if have_xn:
                            nc.vector.tensor_add(t2[:, hs], t1[:, hs], pxn[:, hs])
                        n_src = t2
                    else:
                        n_src = pxn
                    nc.scalar.activation(n_t[:, hs], n_src[:, hs], AF.Tanh)
                    if have_h:
                        weng = nc.gpsimd if CFG["w_on_gpsimd"] else nc.vector
                        weng.tensor_mul(w_t[:, hs], z_t[:, hs], h_prev[:, hs])
                        nc.vector.scalar_tensor_tensor(
                            p_t[:, hs], z_t[:, hs], 1.0, n_t[:, hs], OP.subtract, OP.mult)
                        for q0 in range(hs.start, hs.stop, P):
                            qs = slice(q0, q0 + P)
                            nc.vector.tensor_sub(h_new[:, qs], w_t[:, qs], p_t[:, qs])
                            emit_transp(qs)
                    else:
                        nc.vector.scalar_tensor_tensor(
                            p_t[:, hs], z_t[:, hs], 1.0, n_t[:, hs], OP.subtract, OP.mult)
                        for q0 in range(hs.start, hs.stop, P):
                            qs = slice(q0, q0 + P)
                            nc.vector.tensor_scalar_mul(h_new[:, qs], p_t[:, qs], -1.0)
                            emit_transp(qs)
                h_prev = h_new
                hT_prev = hT_new
                if extra is not None:
                    extra(t)
                if dbg_h is not None:
                    nc.sync.dma_start(dbg_h[1 if is_dec else 0, t], h_new[:])
                if is_dec:
                    prev_h_out = hT_new
            if prev_h_out is not None:
                emit_po(t_steps - 1, prev_h_out)
            return h_prev

        with tc.tile_pool(name="gru_sb", bufs=int(os.environ.get("KCFG_SB_BUFS", 3 if zero_bias else 2))) as sb, \
             tc.tile_pool(name="psum_g", bufs=2, space="PSUM") as pg, \
             tc.tile_pool(name="psum_gx", bufs=1, space="PSUM") as pgx, \
             tc.tile_pool(name="psum_gn", bufs=1, space="PSUM") as pgn, \
             tc.tile_pool(name="psum_gt", bufs=1, space="PSUM") as pgt:
            h_enc = gru_loop("enc", False, None, None, sb, pg, pgx, pgn, pgt, pgo,
                             extra=emb_filler)
            # hidden = h_T + noise (batch-major), then transpose once for
            # the decoder's initial stationary chunks.
            hid = sb.tile([P, H], bf16, tag="h")
            nc.vector.tensor_add(hid[:], h_enc[:], noise_sb[:])
            hidT = sb.tile([P, H], bf16, tag="hT")
            ptr0 = pgt.tile([P, H], bf16, name="ptr0", tag="pt")
            for k in range(4):
                ks = slice(k * P, (k + 1) * P)
                nc.tensor.transpose(ptr0[:, ks], hid[:, ks], ident_bf[:])
                nc.scalar.copy(hidT[:, ks], ptr0[:, ks])
            gru_loop("dec", True, hidT, hid, sb, pg, pgx, pgn, pgt, pgo,
                     extra=emb_filler)

    nc.compile()
    return nc


_CACHE = {}


def _get_module(zero_bias: bool):
    key = zero_bias
    if key not in _CACHE:
        _CACHE[key] = build_module(zero_bias)
    return _CACHE[key]


def kernel(past_input, future_input, noise,
           W_emb, b_emb,
           W_ih_enc, W_hh_enc, b_ih_enc, b_hh_enc,
           W_ih_dec, W_hh_dec, b_ih_dec, b_hh_dec,
           W_out, b_out):
    f = np.float32
    past_input = np.asarray(past_input, f)
    future_input = np.asarray(future_input, f)
    noise = np.asarray(noise, f)
    zero_bias = not any(
        np.any(np.asarray(b)) for b in (b_emb, b_ih_enc, b_hh_enc, b_ih_dec, b_hh_dec, b_out)
    )
    nc = _get_module(zero_bias)

    shared = {
        "w_emb": np.asarray(W_emb, f),
        "b_emb": np.asarray(b_emb, f).reshape(1, E),
        "w_ih_enc": np.asarray(W_ih_enc, f), "w_hh_enc": np.asarray(W_hh_enc, f),
        "b_ih_enc": np.asarray(b_ih_enc, f).reshape(1, H3),
        "b_hh_enc": np.asarray(b_hh_enc, f).reshape(1, H3),
        "w_ih_dec": np.asarray(W_ih_dec, f), "w_hh_dec": np.asarray(W_hh_dec, f),
        "b_ih_dec": np.asarray(b_ih_dec, f).reshape(1, H3),
        "b_hh_dec": np.asarray(b_hh_dec, f).reshape(1, H3),
        "w_out": np.asarray(W_out, f),
        "b_out": np.asarray(b_out, f).reshape(1, D),
    }
    in_maps = []
    for c in range(NCORES):
        sl = slice(c * BL, (c + 1) * BL)
        m = dict(shared)
        m["past"] = np.ascontiguousarray(past_input[:, sl, :])
        m["fut"] = np.ascontiguousarray(future_input[:, sl, :])
        m["noise"] = np.ascontiguousarray(noise[sl, :])
        in_maps.append(m)

    res = bass_utils.run_bass_kernel_spmd(nc, in_maps, core_ids=list(range(NCORES)))
    return np.concatenate([r["out"] for r in res.results], axis=1)


# revision 9
# speedup vs baseline: 1.2327x; 1.2327x over previous
"""Trainium2 Bass kernel for a GRU encoder-decoder (KLCPD generator).

Model (see reference):
  past_emb = relu(past @ W_emb + b_emb)            [T,B,E]
  fut_emb  = relu(future @ W_emb + b_emb)          [T,B,E]
  _, h_T   = GRU_enc(past_emb, h0=0)
  hidden   = h_T + noise
  ys, _    = GRU_dec(shift(fut_emb), h0=hidden)
  out      = ys @ W_out + b_out                    [T,B,D]

Sharding: data-parallel over batch B=1024 across 8 NeuronCores
(B_local=128); all weights replicated; no collectives.

Per-core kernel layout (batch-major / moving-weights scheme):
  * The previous scheme kept the state transposed and used the GRU
    weights as the PE's stationary operand: 72 matmuls per step with
    only N=128 moving columns (the per-core batch).  Measured HW
    profile: 9.6k matmuls at ~79ns each (LDWEIGHTS-bound), PE busy
    82%, 762us total.
  * This scheme flips the operands: the STATE is the stationary
    operand (hT chunks, 4 LDWs/step) and the weights are the MOVING
    operand (W_hh[k] rows streamed as N=512 slices).  12 recurrent +
    6 input-projection matmuls per step, all N=512, running at the
    pure streaming rate (~216ns each) instead of the LDW-bound rate.
  * Gate pre-activations land batch-major ([batch, 3H]) in PSUM:
    banks r,z double-buffered (4), xn single (1), hn single (1),
    h-transpose (1), emb/output-projection shared slot (1) = 8 banks.
  * The tail (sigmoid/tanh/elementwise) is unchanged except that it
    runs batch-major and each 128-col quarter of h_new is immediately
    PE-transposed (bf16 identity) and copied back to SBUF to form the
    next step's stationary hT chunks.
  * Embeddings stream through the loops as before (slot-batched input
    DMAs, PE transposes + matmuls through the shared PSUM bank); embT
    chunks now serve as the stationary operand of the input
    projections with W_ih moving.
  * The decoder output projection reuses the hT chunks as stationary
    (po[b,d] = h @ W_out directly); 4 steps batch into one output DMA.
  * noise no longer needs a transpose (hidden = h_T + noise is
    batch-major).
"""

import os
from contextlib import ExitStack

import numpy as np

import concourse.bass as bass
import concourse.tile as tile
from concourse import bacc, bass_utils, masks, mybir
from concourse.tile_rust import add_dep_helper

T, B, D, E, H = 64, 1024, 128, 256, 512
NCORES = 8
BL = B // NCORES  # 128
H3 = 3 * H
P = 128

f32 = mybir.dt.float32
bf16 = mybir.dt.bfloat16
AF = mybir.ActivationFunctionType
OP = mybir.AluOpType


def _mm(nc, out, lhsT, rhs, start, stop):
    nc.tensor.matmul(out, lhsT, rhs, start=start, stop=stop, skip_group_check=True)


# Tunables.
CFG = {
    "tail_halves": 2,     # 1 = full-width gate ops, 2 = H-halved
    "emb_all_pre": False,  # emit all embedding groups before the enc loop
    "emb_lead": 1,         # steps between emb stage A and stage B
    "emb_e1_gpsimd": False,  # e1 relu on gpsimd instead of DVE
    "emb_cadence": 2,        # steps between embedding-group stage As
    "tail_first_cols": 256,  # width of the first (chain-critical) tail slice
    "po_direct_dma": False,  # accumulate 4 steps in one PSUM bank, DMA direct
    "copy_dve": True,        # hT quarter copies on DVE instead of ACT
}
for _k in list(CFG):
    _v = os.environ.get(f"KCFG_{_k.upper()}")
    if _v is not None:
        CFG[_k] = type(CFG[_k])(int(_v))


def build_module(zero_bias: bool, t_steps: int = T, dump_h: bool = False):
    """Builds the per-core Bass module. Returns the compiled nc."""
    nc = bacc.Bacc("TRN2", target_bir_lowering=False, debug=False)
    dbg_h = None
    if dump_h:
        dbg_h = nc.dram_tensor("dbg_h", [2, t_steps, P, H], bf16, kind="ExternalOutput").ap()

    past = nc.dram_tensor("past", [t_steps, BL, D], f32, kind="ExternalInput").ap()
    fut = nc.dram_tensor("fut", [t_steps, BL, D], f32, kind="ExternalInput").ap()
    noise = nc.dram_tensor("noise", [BL, H], f32, kind="ExternalInput").ap()
    w_emb = nc.dram_tensor("w_emb", [D, E], f32, kind="ExternalInput").ap()
    b_emb = nc.dram_tensor("b_emb", [1, E], f32, kind="ExternalInput").ap()
    wd = {}
    for g in ("enc", "dec"):
        wd[g, "ih"] = nc.dram_tensor(f"w_ih_{g}", [E, H3], f32, kind="ExternalInput").ap()
        wd[g, "hh"] = nc.dram_tensor(f"w_hh_{g}", [H, H3], f32, kind="ExternalInput").ap()
        wd[g, "bih"] = nc.dram_tensor(f"b_ih_{g}", [1, H3], f32, kind="ExternalInput").ap()
        wd[g, "bhh"] = nc.dram_tensor(f"b_hh_{g}", [1, H3], f32, kind="ExternalInput").ap()
    w_out = nc.dram_tensor("w_out", [H, D], f32, kind="ExternalInput").ap()
    b_out = nc.dram_tensor("b_out", [1, D], f32, kind="ExternalInput").ap()
    out = nc.dram_tensor("out", [t_steps, BL, D], f32, kind="ExternalOutput").ap()

    with tile.TileContext(nc, pool_alloc_mode="queue") as tc, ExitStack() as octx:
        wpool = octx.enter_context(tc.tile_pool(name="weights", bufs=1))

        # ---- constants -------------------------------------------------
        ident = wpool.tile([P, P], f32)
        masks.make_identity(nc, ident[:])
        ident_bf = wpool.tile([P, P], bf16)
        masks.make_identity(nc, ident_bf[:])
        ones_row = wpool.tile([1, 512], bf16)
        nc.gpsimd.memset(ones_row[:], 1.0)

        # ---- embedding precompute (streamed) ----------------------------
        # embT[g][e][p, t*BL + b] = relu(x[t] @ W_emb + b_emb)[b, e*128+p]
        embT = {g: [wpool.tile([P, t_steps * BL], bf16, name=f"embT_{g}_{e}", tag=f"embT_{g}_{e}")
                    for e in range(2)]
                for g in ("enc", "dec")}
        n_grp = t_steps // 4
        ep = octx.enter_context(tc.tile_pool(
            name="estage", bufs=int(os.environ.get("KCFG_EP_BUFS", 3 if zero_bias else 1))))
        pgo = octx.enter_context(tc.tile_pool(name="psum_out", bufs=1, space="PSUM"))

        # Slot-batched input loads: ONE DMA covers 4 embedding groups (16
        # timesteps), amortizing the ~625ns per-DMA engine-queue cost.
        GRP_PER_SLOT = int(os.environ.get("KCFG_GRP_PER_SLOT", 4))
        xs_slots = {}

        def emit_xs_slot(g, x_ap, si, split=False):
            lo = si * GRP_PER_SLOT * 4
            hi = min(lo + GRP_PER_SLOT * 4, t_steps)
            xs = ep.tile([P, (hi - lo) * P], f32, tag="xs")
            halves = ((lo, (lo + hi) // 2), ((lo + hi) // 2, hi)) if split else ((lo, hi),)
            for (a, b) in halves:
                nc.sync.dma_start(
                    xs[:, (a - lo) * P:(b - lo) * P].rearrange("p (i d) -> p i d", i=b - a),
                    x_ap[a:b].transpose([1, 0, 2]),
                )
            xs_slots[g, si] = xs

        def emit_emb_stage_a(g, x_ap, gi):
            """Transpose one 4-step group (PE transposes via the shared PSUM
            bank). Returns a closure emitting stage B (matmul + relu)."""
            si, sub = divmod(gi, GRP_PER_SLOT)
            if (g, si) not in xs_slots:
                emit_xs_slot(g, x_ap, si,
                             split=os.environ.get("KCFG_SPLIT_ALL") == "1")
            xs = xs_slots[g, si]
            ptr = pgo.tile([P, 4 * P], f32, tag="po")
            for i in range(4):
                nc.tensor.transpose(ptr[:, i * P:(i + 1) * P],
                                    xs[:, (sub * 4 + i) * P:(sub * 4 + i + 1) * P],
                                    ident[:])
            xT = ep.tile([P, 4 * P], bf16, tag="xT")
            nc.scalar.copy(xT[:], ptr[:])

            def stage_b():
                for e in range(2):
                    pe_ = pgo.tile([P, 4 * P], f32, tag="po")
                    _mm(nc, pe_[:], wemb_bf[:, e * P:(e + 1) * P], xT[:],
                        start=True, stop=zero_bias)
                    if not zero_bias:
                        _mm(nc, pe_[:], bemb_bf[0:1, e * P:(e + 1) * P], ones_row[0:1, :],
                            start=False, stop=True)
                    dst = embT[g][e][:, gi * 4 * P:(gi + 1) * 4 * P]
                    if e == 0:
                        nc.scalar.activation(dst, pe_[:], AF.Relu)
                    elif CFG["emb_e1_gpsimd"]:
                        nc.gpsimd.tensor_scalar_max(dst, pe_[:], 0.0)
                    else:
                        nc.vector.tensor_scalar_max(dst, pe_[:], 0.0)

            return stage_b

        # Highest priority on the sync DMA ring: the first past-input slot.
        emit_xs_slot("enc", past, 0, split=True)

        # ---- weight preload + cast to bf16 -----------------------------
        whh = {}   # whh[g][k]: [128, H3]  (rows k*128..) -- MOVING operand
        wih = {}   # wih[g][e]: [128, H3]  (rows e*128..) -- MOVING operand
        biasx = {}  # [1, H3]  (b_ih + b_hh on r,z cols; b_ih on n cols)
        biashn = {}  # [1, 512] (b_hh n-part)
        stage_ctx = tc.tile_pool(name="wstage", bufs=2)
        stage = octx.enter_context(stage_ctx)

        wemb_bf = wpool.tile([P, E], bf16)
        st = stage.tile([P, E], f32, tag="s_emb")
        nc.sync.dma_start(st[:], w_emb[:, :])
        nc.vector.tensor_copy(wemb_bf[:], st[:])

        bemb_bf = None
        if not zero_bias:
            st = stage.tile([1, E], f32, tag="s_bemb")
            nc.sync.dma_start(st[:], b_emb[:, :])
            bemb_bf = wpool.tile([1, E], bf16)
            nc.vector.tensor_copy(bemb_bf[:], st[:])

        def load_gru_weights(g, cast_engine=None):
            # Encoder weights ride the scalar HWDGE queue so they do not
            # queue behind the input slots on the sync ring (and vice versa).
            dma_eng = nc.scalar if cast_engine is None else nc.sync
            wih[g] = []
            for e in range(2):
                t_ = wpool.tile([P, H3], bf16, tag=f"wih_{g}_{e}")
                st = stage.tile([P, H3], f32, tag="s_ih")
                dma_eng.dma_start(st[:], wd[g, "ih"][e * P:(e + 1) * P, :])
                if cast_engine is not None:
                    cast_engine.tensor_copy(t_[:], st[:])
                elif e % 2 == 0:
                    nc.vector.tensor_copy(t_[:], st[:])
                else:
                    nc.scalar.copy(t_[:], st[:])
                wih[g].append(t_)
            whh[g] = []
            for k in range(4):
                t_ = wpool.tile([P, H3], bf16, tag=f"whh_{g}_{k}")
                st = stage.tile([P, H3], f32, tag="s_hh")
                dma_eng.dma_start(st[:], wd[g, "hh"][k * P:(k + 1) * P, :])
                if cast_engine is None:
                    if k % 2 == 0:
                        nc.vector.tensor_copy(t_[:], st[:])
                    else:
                        nc.scalar.copy(t_[:], st[:])
                else:
                    cast_engine.tensor_copy(t_[:], st[:])
                whh[g].append(t_)
            if not zero_bias:
                sih = stage.tile([1, H3], f32, tag="s_bih")
                shh = stage.tile([1, H3], f32, tag="s_bhh")
                nc.sync.dma_start(sih[:], wd[g, "bih"][:, :])
                nc.sync.dma_start(shh[:], wd[g, "bhh"][:, :])
                bx = wpool.tile([1, H3], bf16, tag=f"biasx_{g}")
                nc.vector.tensor_add(bx[:, 0:2 * H], sih[:, 0:2 * H], shh[:, 0:2 * H])
                nc.vector.tensor_copy(bx[:, 2 * H:H3], sih[:, 2 * H:H3])
                bh = wpool.tile([1, H], bf16, tag=f"biashn_{g}")
                nc.vector.tensor_copy(bh[:], shh[:, 2 * H:H3])
                biasx[g] = bx
                biashn[g] = bh

        load_gru_weights("enc")

        # noise stays batch-major (hidden = h_T + noise needs no transpose)
        noise_sb = wpool.tile([P, H], bf16)
        st = stage.tile([P, H], f32, tag="s_noise")
        nc.sync.dma_start(st[:], noise[:, :])
        nc.vector.tensor_copy(noise_sb[:], st[:])

        wout_bf = wpool.tile([P, 4 * P], bf16)  # col block k = W_out rows k
        st = stage.tile([P, 4 * P], f32, tag="s_out")
        for k in range(4):
            nc.sync.dma_start(st[:, k * P:(k + 1) * P], w_out[k * P:(k + 1) * P, :])
        nc.vector.tensor_copy(wout_bf[:], st[:])
        if not zero_bias:
            bout_bf = wpool.tile([1, D], bf16)
            st = stage.tile([1, D], f32, tag="s_bout")
            nc.sync.dma_start(st[:], b_out[:, :])
            nc.vector.tensor_copy(bout_bf[:], st[:])

        for _si in range(1, (n_grp + GRP_PER_SLOT - 1) // GRP_PER_SLOT):
            emit_xs_slot("enc", past, _si,
                         split=os.environ.get("KCFG_SPLIT_ALL") == "1")

        N_PRE = n_grp if CFG["emb_all_pre"] else int(os.environ.get("KCFG_N_PRE", 3))
        for gi in range(N_PRE):
            emit_emb_stage_a("enc", past, gi)()
        if CFG["emb_all_pre"]:
            for gi in range(n_grp):
                emit_emb_stage_a("dec", fut, gi)()

        emb_jobs = [("enc", past, gi) for gi in range(N_PRE, n_grp)]
        if not CFG["emb_all_pre"]:
            emb_jobs += [("dec", fut, gi) for gi in range(n_grp)]
        emb_jobs.reverse()  # consume via pop()
        emb_pending = []   # list of (due_step, stage_b)
        emb_clock = [0, -10]  # [global step counter, last stage-A step]

        def emb_filler(t):
            tc_ = emb_clock[0]
            emb_clock[0] += 1
            while emb_pending and emb_pending[0][0] <= tc_:
                emb_pending.pop(0)[1]()
            if (not emb_pending and emb_jobs
                    and tc_ - emb_clock[1] >= CFG["emb_cadence"]):
                emb_clock[1] = tc_
                emb_pending.append((tc_ + CFG["emb_lead"], emit_emb_stage_a(*emb_jobs.pop())))

        # Decoder weights: casts on the otherwise-idle GpSimd engine.
        load_gru_weights("dec", cast_engine=nc.gpsimd)

        # ---- GRU loops --------------------------------------------------
        def gru_loop(g, is_dec, hT0, h0_sb, sb, pg, pgx, pgn, pgt, pgo, extra=None):
            """Runs t_steps of GRU g in batch-major layout.
            hT0 = initial transposed state chunks (SBUF [P, H] bf16,
            hT[p, k*P+b] = h[b, k*P+p]) or None; h0_sb = same state
            batch-major or None. Returns the final batch-major state."""
            hT_prev = hT0
            h_prev = h0_sb

            def alloc_xw(t):
                """Allocate step t's r/z/xn PSUM banks. Returns a state dict;
                the input-projection matmuls are emitted later, one embedding
                chunk at a time, via emit_x_mms (PE filler placement)."""
                have_x = (not is_dec) or t > 0
                have_h = t > 0 or hT0 is not None
                have_xn = have_x or not zero_bias
                pr = pg.tile([P, H], f32, name="pr", tag="pr")
                pz = pg.tile([P, H], f32, name="pz", tag="pz")
                pxn = pgx.tile([P, H], f32, name="pxn", tag="pxn") if have_xn else None

                nbias = 0 if zero_bias else 1
                nxw = (2 if have_x else 0) + nbias
                nhw = 4 if have_h else 0
                totals = {id(pr): nxw + nhw, id(pz): nxw + nhw}
                if pxn is not None:
                    totals[id(pxn)] = nxw
                emitted = {k: 0 for k in totals}
                return {"t": t, "pr": pr, "pz": pz, "pxn": pxn,
                        "have_x": have_x, "totals": totals, "emitted": emitted}

            def mk_emit(st):
                totals, emitted = st["totals"], st["emitted"]

                def emit(bank, lhsT, rhs):
                    emitted[id(bank)] += 1
                    _mm(nc, bank[:], lhsT, rhs,
                        start=emitted[id(bank)] == 1,
                        stop=emitted[id(bank)] == totals[id(bank)])
                return emit

            def emit_x_mms(st, e):
                """Emit the input projections of embedding chunk e for the
                step of state `st` (3 matmuls, W_ih moving)."""
                if not st["have_x"]:
                    return
                emit = mk_emit(st)
                t = st["t"]
                tcol = (t - 1) if is_dec else t
                lx = embT[g][e][:, tcol * BL:(tcol + 1) * BL]
                for bank, lo in ((st["pr"], 0), (st["pz"], H), (st["pxn"], 2 * H)):
                    if bank is None:
                        continue
                    emit(bank, lx, wih[g][e][:, lo:lo + H])

            def emit_x_bias(st):
                if zero_bias:
                    return
                emit = mk_emit(st)
                for bank, lo in ((st["pr"], 0), (st["pz"], H), (st["pxn"], 2 * H)):
                    if bank is None:
                        continue
                    emit(bank, ones_row[0:1, 0:P], biasx[g][0:1, lo:lo + H])

            po_buf = [None]

            def emit_po(t, hT):
                """Output projection of decoder step t: out[t] = h @ W_out
                (+ b_out), with hT chunks stationary and W_out rows moving;
                lands batch-major [b, d]. 4 steps per output DMA."""
                sub = t % 4
                if CFG["po_direct_dma"]:
                    if sub == 0:
                        po_buf[0] = pgo.tile([P, 512], f32, name="po", tag="po")
                    po = po_buf[0]
                    sl = po[:, sub * P:(sub + 1) * P]
                    if not zero_bias:
                        _mm(nc, sl, ones_row[0:1, 0:P], bout_bf[0:1, :],
                            start=sub == 0, stop=False)
                    for k in range(4):
                        _mm(nc, sl, hT[:, k * P:(k + 1) * P], wout_bf[:, k * P:(k + 1) * P],
                            start=zero_bias and sub == 0 and k == 0, stop=k == 3)
                    if sub == 3:
                        outf = sb.tile([P, 4 * P], f32, name="outf", tag="outf")
                        nc.scalar.copy(outf[:], po[:])
                        nc.sync.dma_start(
                            out[t - 3:t + 1].transpose([1, 0, 2]),
                            outf[:].rearrange("p (i d) -> p i d", i=4),
                        )
                    return
                po = pgo.tile([P, 512], f32, name="po", tag="po")
                sl = po[:, 0:P]
                if not zero_bias:
                    _mm(nc, sl, ones_row[0:1, 0:P], bout_bf[0:1, :],
                        start=True, stop=False)
                for k in range(4):
                    _mm(nc, sl, hT[:, k * P:(k + 1) * P], wout_bf[:, k * P:(k + 1) * P],
                        start=zero_bias and k == 0, stop=k == 3)
                if po_buf[0] is None:
                    po_buf[0] = sb.tile([P, 4 * P], f32, name="outf", tag="outf")
                nc.scalar.copy(po_buf[0][:, sub * P:(sub + 1) * P], sl)
                if sub == 3:
                    nc.sync.dma_start(
                        out[t - 3:t + 1].transpose([1, 0, 2]),
                        po_buf[0][:].rearrange("p (i d) -> p i d", i=4),
                    )
                    po_buf[0] = None

            prev_h_out = None
            state = alloc_xw(0)
            emit_x_bias(state)
            emit_x_mms(state, 0)
            emit_x_mms(state, 1)
            for t in range(t_steps):
                have_x = (not is_dec) or t > 0
                have_h = hT_prev is not None
                have_xn = have_x or not zero_bias
                st = state
                pr, pz, pxn = st["pr"], st["pz"], st["pxn"]
                emit = mk_emit(st)

                # -- recurrent matmuls: stationary hT(t-1) chunk k, moving
                # W_hh rows. ALL 12 go first, bank-major (r, hn, z): the
                # tail cannot start an op until its bank's last matmul. --
                phn = None
                if have_h:
                    phn = pgn.tile([P, H], f32, name="phn", tag="pn")
                    st["totals"][id(phn)] = 4 + (0 if zero_bias else 1)
                    st["emitted"][id(phn)] = 0
                    for k in range(4):
                        emit(pr, hT_prev[:, k * P:(k + 1) * P],
                             whh[g][k][:, 0:H])
                    if not zero_bias:
                        emit(phn, ones_row[0:1, 0:P], biashn[g][0:1, :])
                    for k in range(4):
                        emit(phn, hT_prev[:, k * P:(k + 1) * P],
                             whh[g][k][:, 2 * H:H3])
                    for k in range(4):
                        emit(pz, hT_prev[:, k * P:(k + 1) * P],
                             whh[g][k][:, H:2 * H])

                # -- gate math (batch-major), chunked in halves.
                #    h = z*(h_prev - n) + n: after sigmoid(z) only two DVE
                #    ops remain on the chain to each h quarter. ------------
                r_t = sb.tile([P, H], bf16, name="r_t", tag="r")
                z_t = sb.tile([P, H], bf16, name="z_t", tag="z")
                n_t = sb.tile([P, H], bf16, tag="n")
                m_t = sb.tile([P, H], bf16, tag="m")
                h_new = sb.tile([P, H], bf16, tag="h")
                # Last encoder step: hT unused (hidden-add is batch-major).
                skip_T = (not is_dec) and t == t_steps - 1
                hT_new = None
                ptr_t = None
                if not skip_T:
                    hT_new = sb.tile([P, H], bf16, name="hT_new", tag="hT")
                    ptr_t = pgt.tile([P, H], bf16, name="ptr_t", tag="pt")
                if have_h:
                    t1 = sb.tile([P, H], bf16, tag="t1")
                    t2 = sb.tile([P, H], bf16, name="t2", tag="t2") if have_xn else t1
                    d_t = sb.tile([P, H], bf16, tag="d")

                fc = CFG["tail_first_cols"]
                tail_slices = ([slice(0, fc), slice(fc, H)] if CFG["tail_halves"] == 2
                               else [slice(0, H)])
                for hs in tail_slices:
                    nc.scalar.activation(r_t[:, hs], pr[:, hs], AF.Sigmoid)
                    if have_h:
                        nc.vector.tensor_mul(t1[:, hs], r_t[:, hs], phn[:, hs])
                        if have_xn:
                            nc.vector.tensor_add(t2[:, hs], t1[:, hs], pxn[:, hs])
                        n_src = t2
                    else:
                        n_src = pxn
                    nc.scalar.activation(n_t[:, hs], n_src[:, hs], AF.Tanh)
                    if have_h:
                        # d is off the sigmoid(z) chain (needs tanh only)
                        nc.vector.tensor_sub(d_t[:, hs], h_prev[:, hs], n_t[:, hs])
                    nc.scalar.activation(z_t[:, hs], pz[:, hs], AF.Sigmoid)
                    if have_h:
                        nc.vector.tensor_mul(m_t[:, hs], z_t[:, hs], d_t[:, hs])
                        for q0 in range(hs.start, hs.stop, P):
                            qs = slice(q0, q0 + P)
                            nc.vector.tensor_add(h_new[:, qs], m_t[:, qs], n_t[:, qs])
                    else:
                        nc.vector.tensor_mul(m_t[:, hs], z_t[:, hs], n_t[:, hs])
                        for q0 in range(hs.start, hs.stop, P):
                            qs = slice(q0, q0 + P)
                            nc.vector.tensor_sub(h_new[:, qs], n_t[:, qs], m_t[:, qs])

                # -- PE fillers + transposes, ordered so the PE reaches each
                # transpose after its h quarter exists: po / xw(t+1) chunk 0
                # / transp q0,q1 / xw chunk 1 / transp q2,q3 / copies. ------
                if prev_h_out is not None:
                    emit_po(t - 1, prev_h_out)
                    prev_h_out = None
                if t + 1 < t_steps:
                    state = alloc_xw(t + 1)
                    emit_x_bias(state)
                    emit_x_mms(state, 0)

                def emit_transp(qs):
                    if hT_new is not None:
                        nc.tensor.transpose(ptr_t[:, qs], h_new[:, qs], ident_bf[:])

                def emit_copy(qs):
                    if hT_new is None:
                        return
                    if CFG["copy_dve"]:
                        nc.vector.tensor_copy(hT_new[:, qs], ptr_t[:, qs])
                    else:
                        nc.scalar.copy(hT_new[:, qs], ptr_t[:, qs])

                emit_transp(slice(0, P))
                emit_transp(slice(P, 2 * P))
                if t + 1 < t_steps:
                    emit_x_mms(state, 1)
                emit_transp(slice(2 * P, 3 * P))
                emit_transp(slice(3 * P, 4 * P))
                for q in range(4):
                    emit_copy(slice(q * P, (q + 1) * P))

                h_prev = h_new
                hT_prev = hT_new
                if extra is not None:
                    extra(t)
                if dbg_h is not None:
                    nc.sync.dma_start(dbg_h[1 if is_dec else 0, t], h_new[:])
                if is_dec:
                    prev_h_out = hT_new
            if prev_h_out is not None:
                emit_po(t_steps - 1, prev_h_out)
            return h_prev

        with tc.tile_pool(name="gru_sb", bufs=int(os.environ.get("KCFG_SB_BUFS", 3 if zero_bias else 2))) as sb, \
             tc.tile_pool(name="psum_g", bufs=2, space="PSUM") as pg, \
             tc.tile_pool(name="psum_gx", bufs=1, space="PSUM") as pgx, \
             tc.tile_pool(name="psum_gn", bufs=1, space="PSUM") as pgn, \
             tc.tile_pool(name="psum_gt", bufs=1, space="PSUM") as pgt:
            h_enc = gru_loop("enc", False, None, None, sb, pg, pgx, pgn, pgt, pgo,
                             extra=emb_filler)
            # hidden = h_T + noise (batch-major), then transpose once for
            # the decoder's initial stationary chunks.
            hid = sb.tile([P, H], bf16, tag="h")
            nc.vector.tensor_add(hid[:], h_enc[:], noise_sb[:])
            hidT = sb.tile([P, H], bf16, tag="hT")
            ptr0 = pgt.tile([P, H], bf16, name="ptr0", tag="pt")
            for k in range(4):
                ks = slice(k * P, (k + 1) * P)
                nc.tensor.transpose(ptr0[:, ks], hid[:, ks], ident_bf[:])
                nc.scalar.copy(hidT[:, ks], ptr0[:, ks])
            gru_loop("dec", True, hidT, hid, sb, pg, pgx, pgn, pgt, pgo,
                     extra=emb_filler)

    nc.compile()
    return nc


_CACHE = {}


def _get_module(zero_bias: bool):
    key = zero_bias
    if key not in _CACHE:
        _CACHE[key] = build_module(zero_bias)
    return _CACHE[key]


def kernel(past_input, future_input, noise,
           W_emb, b_emb,
           W_ih_enc, W_hh_enc, b_ih_enc, b_hh_enc,
           W_ih_dec, W_hh_dec, b_ih_dec, b_hh_dec,
           W_out, b_out):
    f = np.float32
    past_input = np.asarray(past_input, f)
    future_input = np.asarray(future_input, f)
    noise = np.asarray(noise, f)
    zero_bias = not any(
        np.any(np.asarray(b)) for b in (b_emb, b_ih_enc, b_hh_enc, b_ih_dec, b_hh_dec, b_out)
    )
    nc = _get_module(zero_bias)

    shared = {
        "w_emb": np.asarray(W_emb, f),
        "b_emb": np.asarray(b_emb, f).reshape(1, E),
        "w_ih_enc": np.asarray(W_ih_enc, f), "w_hh_enc": np.asarray(W_hh_enc, f),
        "b_ih_enc": np.asarray(b_ih_enc, f).reshape(1, H3),
        "b_hh_enc": np.asarray(b_hh_enc, f).reshape(1, H3),
        "w_ih_dec": np.asarray(W_ih_dec, f), "w_hh_dec": np.asarray(W_hh_dec, f),
        "b_ih_dec": np.asarray(b_ih_dec, f).reshape(1, H3),
        "b_hh_dec": np.asarray(b_hh_dec, f).reshape(1, H3),
        "w_out": np.asarray(W_out, f),
        "b_out": np.asarray(b_out, f).reshape(1, D),
    }
    in_maps = []
    for c in range(NCORES):
        sl = slice(c * BL, (c + 1) * BL)
        m = dict(shared)
        m["past"] = np.ascontiguousarray(past_input[:, sl, :])
        m["fut"] = np.ascontiguousarray(future_input[:, sl, :])
        m["noise"] = np.ascontiguousarray(noise[sl, :])
        in_maps.append(m)

    res = bass_utils.run_bass_kernel_spmd(nc, in_maps, core_ids=list(range(NCORES)))
    return np.concatenate([r["out"] for r in res.results], axis=1)
